# revision 11
# baseline (speedup 1.0000x reference)
"""PolyGCL GNN message-passing kernel for 8 Trainium2 NeuronCores.

Strategy (node sharding, hint-aligned):
  - Nodes are range-sharded across the 8 cores (shard = N/8 rows, padded to a
    multiple of 128). Per-node state is kept in the transformed space
    u = D^{-1/2} h, which makes the GCN edge weight separable:
        msg_u[d] = sum_{e: dst=d} u[src_e]          (pure 0/1 segment sum)
        adj_u(u) = invdeg * (msg_u + u)
    so the scatter matmul needs only one-hot selection tiles (no per-edge w).
  - Each hop: AllGather the bf16 u-shards into a full HBM table, dma_gather
    256B rows per edge (edge-major tiles), one-hot segment-sum matmuls on the
    tensor engine into PSUM per 128-dst window, then a fused combine on DVE.
  - Edges are bucketed by (dst window of 128, src quarter) on the host; the
    src quarter keeps gather indices within int16 range. Tile counts per
    bucket are maxed across cores so all 8 cores run one SPMD program.
  - BatchNorm uses local per-feature sums + a tiny AllReduce; encoder and
    output Linear layers are data-parallel over node shards.
"""

import numpy as np
import ml_dtypes

import concourse.bass as bass
import concourse.bacc as bacc
import concourse.tile as tile
from concourse import mybir
from concourse.masks import make_identity

F32 = mybir.dt.float32
BF16 = mybir.dt.bfloat16
I16 = mybir.dt.int16

CFG = dict(N=100000, E=1600000, IN=512, H=128, OUT=128, K=10, C=8,
           BN_EPS=1e-5)

CALLT = 16     # tiles (of 128 edges) per dma_gather call
SCH = 16       # tiles per one-hot S-build batch
NPASS = 4      # src quarters (int16 gather index range)
PAD_DSTL = 300.0  # out-of-window sentinel for padded edge slots


# --------------------------------------------------------------------------
# Host-side schedule
# --------------------------------------------------------------------------

def make_schedule(edge_index, N, C):
    """Bucket edges by (core, dst-window, src-quarter), pad each bucket to a
    multiple of 128 using the max count across cores (one shared SPMD
    schedule), and emit per-core gather-index / dst-slot arrays."""
    src = np.asarray(edge_index[0], dtype=np.int64)
    dst = np.asarray(edge_index[1], dtype=np.int64)
    SHARD = N // C
    NT = -(-SHARD // 128)          # node tiles == dst windows per core
    SHARDP = NT * 128
    QSP = SHARDP * C // NPASS      # quarter size in padded-global ids
    assert QSP <= 32768

    core = dst // SHARD
    local = dst - core * SHARD
    w = local // 128
    dslot = local - w * 128
    srcp = (src // SHARD) * SHARDP + (src % SHARD)   # padded-global id
    q = srcp // QSP
    idxval = srcp - q * QSP

    # counts per (core, w, q)
    key = (core * NT + w) * NPASS + q
    counts = np.bincount(key, minlength=C * NT * NPASS).reshape(C, NT, NPASS)
    T = -(-counts.max(axis=0) // 128)                 # [NT, NPASS] tiles
    nowin = T.sum(axis=1) == 0
    T[nowin, 0] = 1
    TT = int(T.sum())

    # base slot (in edges) of each (w, q) bucket, w-major q-minor
    sizes = (T * 128).reshape(-1)
    base = np.zeros(NT * NPASS, dtype=np.int64)
    base[1:] = np.cumsum(sizes)[:-1]
    base = base.reshape(NT, NPASS)

    # per-tile metadata in global (w-major, q-minor) order
    tile_w = np.repeat(np.arange(NT), T.sum(axis=1))
    tile_q = np.concatenate([np.repeat(np.arange(NPASS), T[wi]) for wi in range(NT)])
    # pass streams: position of each global tile within its pass
    pos_in_pass = np.zeros(TT, dtype=np.int64)
    for qq in range(NPASS):
        m = tile_q == qq
        pos_in_pass[m] = np.arange(m.sum())
    n_tiles_pass = [int((tile_q == qq).sum()) for qq in range(NPASS)]

    # per-core padded slot arrays
    order = np.lexsort((q, w, core))
    so_core, so_w, so_q = core[order], w[order], q[order]
    so_idx, so_dslot = idxval[order], dslot[order]
    skey = (so_core * NT + so_w) * NPASS + so_q
    # rank within each (c,w,q) group
    group_change = np.empty(len(skey), dtype=bool)
    group_change[0] = True
    group_change[1:] = skey[1:] != skey[:-1]
    gstart = np.where(group_change)[0]
    gid = np.cumsum(group_change) - 1
    rank = np.arange(len(skey)) - gstart[gid]
    slot = base[so_w, so_q] + rank

    idx_slots = np.zeros((C, TT * 128), dtype=np.int16)
    dstl_slots = np.full((C, TT * 128), PAD_DSTL, dtype=np.float32)
    idx_slots[so_core, slot] = so_idx.astype(np.int16)
    dstl_slots[so_core, slot] = so_dslot.astype(np.float32)

    # wrapped int16 index arrays per pass, per core: [128, L/16]
    idx_by_tile = idx_slots.reshape(C, TT, 128)
    idx_pass = []
    for qq in range(NPASS):
        sel = idx_by_tile[:, tile_q == qq, :].reshape(C, -1)   # [C, Lq]
        Lq = sel.shape[1]
        wrapped = sel.reshape(C, Lq // 16, 16).transpose(0, 2, 1)  # [C,16,Lq/16]
        idx_pass.append(np.tile(wrapped, (1, 8, 1)).copy())        # [C,128,..]

    dstl = np.ascontiguousarray(
        dstl_slots.reshape(C, TT, 128).transpose(0, 2, 1)
    ).astype(ml_dtypes.bfloat16)                                   # [C,128,TT]

    # window -> (first_tile, last_tile) global ids; tiles of w are contiguous
    win_first = np.zeros(NT, dtype=np.int64)
    win_last = np.zeros(NT, dtype=np.int64)
    tstart = 0
    for wi in range(NT):
        n = int(T[wi].sum())
        win_first[wi] = tstart
        win_last[wi] = tstart + n - 1
        tstart += n

    return dict(
        SHARD=SHARD, NT=NT, SHARDP=SHARDP, QSP=QSP, TT=TT,
        tile_w=tile_w, tile_q=tile_q, pos_in_pass=pos_in_pass,
        n_tiles_pass=n_tiles_pass, win_first=win_first, win_last=win_last,
        idx_pass=idx_pass, dstl=dstl,
    )


# --------------------------------------------------------------------------
# BIR builder (one SPMD program for all 8 cores)
# --------------------------------------------------------------------------

def build(sched, cfg, gammas, alpha, high_pass, n_hops=None,
          skip_collective=False, skip_gather=False, skip_matmul=False,
          skip_sbuild=False):
    N, IN, H, OUT, C = cfg["N"], cfg["IN"], cfg["H"], cfg["OUT"], cfg["C"]
    K = cfg["K"] if n_hops is None else n_hops
    EPS = cfg["BN_EPS"]
    SHARD, NT, SHARDP, QSP, TT = (sched[k] for k in
                                  ("SHARD", "NT", "SHARDP", "QSP", "TT"))
    tile_w, tile_q = sched["tile_w"], sched["tile_q"]
    pos_in_pass = sched["pos_in_pass"]
    n_tiles_pass = sched["n_tiles_pass"]
    win_first, win_last = sched["win_first"], sched["win_last"]
    KC = IN // 128
    gam = [float(g) for g in gammas]
    alpha = float(alpha)

    nc = bacc.Bacc("TRN2", target_bir_lowering=False, debug=False,
                   num_devices=C, enable_asserts=False,
                   num_swdge_queues=4, dynamic_dma_scratch_size=24576)

    # ---- I/O ----
    xT_in = nc.dram_tensor("xT", [IN, SHARDP], BF16, kind="ExternalInput")
    wencT_in = nc.dram_tensor("wencT", [IN, H], BF16, kind="ExternalInput")
    benc_in = nc.dram_tensor("benc_b", [128, H], F32, kind="ExternalInput")
    wupT_in = nc.dram_tensor("wupT", [H, OUT], BF16, kind="ExternalInput")
    bup_in = nc.dram_tensor("bup", [OUT, 1], F32, kind="ExternalInput")
    bng_in = nc.dram_tensor("bng", [H, 1], F32, kind="ExternalInput")
    bnb_in = nc.dram_tensor("bnb", [H, 1], F32, kind="ExternalInput")
    invdeg_in = nc.dram_tensor("invdeg", [128, NT], F32, kind="ExternalInput")
    isqd_in = nc.dram_tensor("isqd", [128, NT], F32, kind="ExternalInput")
    sqd_in = nc.dram_tensor("sqd", [128, NT], F32, kind="ExternalInput")
    dstl_in = nc.dram_tensor("dstl", [128, TT], BF16, kind="ExternalInput")
    idx_ins = [
        nc.dram_tensor(f"idx{qq}", [128, max(n_tiles_pass[qq] * 8, 1)], I16,
                       kind="ExternalInput")
        for qq in range(NPASS)
    ]
    seed_in = nc.dram_tensor("seed", [1, 1], F32, kind="ExternalInput")
    out_t = nc.dram_tensor("out", [SHARD, OUT], F32, kind="ExternalOutput")
    seed_out = nc.dram_tensor("seed_out", [1, 1], F32, kind="ExternalOutput")

    FT = SHARD // 128          # full node tiles
    REM = SHARD - FT * 128     # rows in the partial tile (0 => exact)

    with tile.TileContext(nc) as tc:
        with (
            tc.tile_pool(name="state", bufs=1) as state,
            tc.tile_pool(name="dram", bufs=1, space="DRAM") as dram,
            tc.tile_pool(name="mbuf", bufs=2) as mbuf,
            tc.tile_pool(name="sbld", bufs=2) as sbld,
            tc.tile_pool(name="psum", bufs=4, space="PSUM") as psum,
            tc.tile_pool(name="tmp", bufs=1) as tmp,
        ):
            # ---- persistent state ----
            u = state.tile([128, NT, H], F32, tag="u")
            acc = state.tile([128, NT, H], F32, tag="acc")
            dstl = state.tile([128, TT], BF16, tag="dstl")
            idxs = [state.tile([128, max(n_tiles_pass[qq] * 8, 1)], I16,
                               tag=f"idx{qq}", name=f"idx{qq}_sb")
                    for qq in range(NPASS)]
            iota_bf = state.tile([128, 128], BF16, tag="iota")
            iota_f = state.tile([128, 128], F32, tag="iotaf")
            ident = state.tile([128, 128], F32, tag="ident")
            benc = state.tile([128, H], F32, tag="benc")
            wencT = state.tile([128, KC, H], BF16, tag="wencT")
            wupT = state.tile([128, OUT], BF16, tag="wupT")
            bup = state.tile([128, 1], F32, tag="bup")
            bng = state.tile([128, 1], F32, tag="bng")
            bnb = state.tile([128, 1], F32, tag="bnb")
            invdeg = state.tile([128, NT], F32, tag="invdeg")
            isqd = state.tile([128, NT], F32, tag="isqd")
            sqd = state.tile([128, NT], F32, tag="sqd")

            ag_in = dram.tile([SHARDP, H], BF16)
            ag_out = dram.tile([SHARDP * C, H], BF16)
            stats_in = dram.tile([H, 2], F32)
            stats_out = dram.tile([H, 2], F32)

            # ---- loads / constants ----
            nc.sync.dma_start(dstl[:], dstl_in[:])
            for qq in range(NPASS):
                nc.sync.dma_start(idxs[qq][:], idx_ins[qq][:])
            nc.sync.dma_start(benc[:], benc_in[:])
            nc.sync.dma_start(
                wencT[:], wencT_in[:].rearrange("(kc p) h -> p kc h", p=128))
            nc.sync.dma_start(wupT[:], wupT_in[:])
            nc.sync.dma_start(bup[:], bup_in[:])
            nc.sync.dma_start(bng[:], bng_in[:])
            nc.sync.dma_start(bnb[:], bnb_in[:])
            nc.sync.dma_start(invdeg[:], invdeg_in[:])
            nc.sync.dma_start(isqd[:], isqd_in[:])
            nc.sync.dma_start(sqd[:], sqd_in[:])
            nc.gpsimd.iota(iota_f[:], pattern=[[1, 128]], base=0,
                           channel_multiplier=0,
                           allow_small_or_imprecise_dtypes=True)
            nc.vector.tensor_copy(iota_bf[:], iota_f[:])
            make_identity(nc, ident[:])

            # seed passthrough (timing hook)
            sd = state.tile([1, 1], F32, tag="seed")
            nc.sync.dma_start(sd[:], seed_in[:])
            nc.scalar.add(sd[:], sd[:], 1.0)
            nc.sync.dma_start(seed_out[:], sd[:])

            ag_in_t = ag_in[:].rearrange("(t p) h -> p t h", p=128)

            def stage_group(g0, n):
                """u[:, g0:g0+n, :] -> ag_in (fp32 -> bf16 cast DMA)."""
                nc.gpsimd.dma_start(
                    ag_in_t[:, g0:g0 + n, :], u[:, g0:g0 + n, :])

            # ---- encoder: u0 = (x @ W_enc.T + b_enc) * isqd ----
            XCH = 2
            xT_r = xT_in[:].rearrange("(kc p) n -> p kc n", p=128)
            ps_enc = None
            for t in range(NT):
                if t % XCH == 0:
                    nx = min(XCH, NT - t) * 128
                    xc = mbuf.tile([128, KC, XCH * 128], BF16, tag="xchunk")
                    nc.sync.dma_start(
                        xc[:, :, :nx], xT_r[:, :, t * 128: t * 128 + nx])
                pt = t % 4
                if pt == 0:
                    ps_enc = psum.tile([128, 512], F32, tag="ps")
                for kc in range(KC):
                    nc.tensor.matmul(
                        ps_enc[:, pt * 128:(pt + 1) * 128],
                        lhsT=xc[:, kc, (t % XCH) * 128:(t % XCH) * 128 + 128],
                        rhs=wencT[:, kc, :],
                        start=(kc == 0), stop=(kc == KC - 1))
                if pt == 3 or t == NT - 1:
                    g0 = t - pt
                    n = pt + 1
                    psv = ps_enc[:, :n * 128].rearrange(
                        "p (t h) -> p t h", h=128)
                    t1 = tmp.tile([128, 4, 128], F32, tag="t1")
                    nc.vector.tensor_tensor(
                        out=t1[:, :n, :], in0=psv,
                        in1=benc[:].rearrange("p (o h) -> p o h", o=1)
                        .to_broadcast([128, n, 128]),
                        op=mybir.AluOpType.add)
                    nc.vector.tensor_tensor(
                        out=u[:, g0:g0 + n, :], in0=t1[:, :n, :],
                        in1=isqd[:, g0:g0 + n].to_broadcast([128, n, 128]),
                        op=mybir.AluOpType.mult)
                    nc.vector.tensor_scalar(
                        out=acc[:, g0:g0 + n, :], in0=u[:, g0:g0 + n, :],
                        scalar1=gam[0], scalar2=None,
                        op0=mybir.AluOpType.mult)
                    if K > 0:
                        stage_group(g0, n)

            # ---- hops ----
            for k in range(1, K + 1):
                if not skip_collective:
                    nc.gpsimd.collective_compute(
                        "AllGather", mybir.AluOpType.bypass,
                        replica_groups=[list(range(C))],
                        ins=[ag_in.opt()], outs=[ag_out.opt()],
                    )
                calls_emitted = [0] * NPASS
                mtiles = [None] * NPASS   # current gather output tile per pass
                msizes = [0] * NPASS
                sch_tile = None
                ps = None
                for t in range(TT):
                    wq = int(tile_q[t])
                    wi = int(tile_w[t])
                    j = int(pos_in_pass[t])
                    # gather call covering pass-tile j
                    if j // CALLT >= calls_emitted[wq]:
                        cid = calls_emitted[wq]
                        j0 = cid * CALLT
                        ntile = min(CALLT, n_tiles_pass[wq] - j0)
                        mt = mbuf.tile([128, CALLT, H], BF16, tag=f"m{wq}")
                        if not skip_gather:
                            nc.gpsimd.dma_gather(
                                out_ap=mt[:, :ntile, :],
                                in_ap=ag_out[wq * QSP:(wq + 1) * QSP, :],
                                idxs_ap=idxs[wq][:, j0 * 8: j0 * 8 + ntile * 8],
                                num_idxs=ntile * 128,
                                num_idxs_reg=ntile * 128,
                                elem_size=H,
                                single_packet=False,
                                queue_num=wq,
                            )
                        else:
                            # ablation: 1-tile gather, matmuls read slot 0
                            nc.gpsimd.dma_gather(
                                out_ap=mt[:, :1, :],
                                in_ap=ag_out[wq * QSP:(wq + 1) * QSP, :],
                                idxs_ap=idxs[wq][:, j0 * 8: j0 * 8 + 8],
                                num_idxs=128,
                                num_idxs_reg=128,
                                elem_size=H,
                                single_packet=False,
                                queue_num=wq,
                            )
                        mtiles[wq] = mt
                        msizes[wq] = ntile
                        calls_emitted[wq] += 1
                    # S chunk build
                    if t % SCH == 0:
                        nsc = min(SCH, TT - t) if not skip_sbuild else 1
                        sch_tile = sbld.tile([128, SCH, 128], BF16, tag="s")
                        nc.vector.tensor_tensor(
                            out=sch_tile[:, :nsc, :],
                            in0=dstl[:, t:t + nsc].to_broadcast([128, nsc, 128]),
                            in1=iota_bf[:].rearrange("p (o h) -> p o h", o=1)
                            .to_broadcast([128, nsc, 128]),
                            op=mybir.AluOpType.is_equal)
                    if wi % 4 == 0 and t == win_first[wi]:
                        ps = psum.tile([128, 512], F32, tag="ps")
                    sl = (wi % 4) * 128
                    s_slot = (t % SCH) if not skip_sbuild else 0
                    m_slot = (j % CALLT) if not skip_gather else 0
                    if not skip_matmul:
                        nc.tensor.matmul(
                            ps[:, sl:sl + 128],
                            lhsT=sch_tile[:, s_slot, :],
                            rhs=mtiles[wq][:, m_slot, :],
                            start=(t == win_first[wi]),
                            stop=(t == win_last[wi]))
                    elif t == win_first[wi]:
                        nc.tensor.matmul(
                            ps[:, sl:sl + 128],
                            lhsT=sch_tile[:, s_slot, :],
                            rhs=mtiles[wq][:, m_slot, :],
                            start=True, stop=True)
                    # combine at group end
                    if t == win_last[wi] and (wi % 4 == 3 or wi == NT - 1):
                        g0 = (wi // 4) * 4
                        n = wi - g0 + 1
                        psv = ps[:, :n * 128].rearrange(
                            "p (t h) -> p t h", h=128)
                        uv = u[:, g0:g0 + n, :]
                        t1 = tmp.tile([128, 4, 128], F32, tag="t1")
                        nc.vector.tensor_tensor(
                            out=t1[:, :n, :], in0=psv, in1=uv,
                            op=mybir.AluOpType.add)
                        nc.vector.tensor_tensor(
                            out=t1[:, :n, :], in0=t1[:, :n, :],
                            in1=invdeg[:, g0:g0 + n].to_broadcast(
                                [128, n, 128]),
                            op=mybir.AluOpType.mult)
                        if high_pass:
                            nc.vector.tensor_tensor(
                                out=uv, in0=uv, in1=t1[:, :n, :],
                                op=mybir.AluOpType.subtract)
                        else:
                            nc.vector.tensor_copy(uv, t1[:, :n, :])
                        t2 = tmp.tile([128, 4, 128], F32, tag="t2")
                        nc.vector.tensor_scalar(
                            out=t2[:, :n, :], in0=uv, scalar1=gam[k],
                            scalar2=None, op0=mybir.AluOpType.mult)
                        nc.vector.tensor_tensor(
                            out=acc[:, g0:g0 + n, :],
                            in0=acc[:, g0:g0 + n, :], in1=t2[:, :n, :],
                            op=mybir.AluOpType.add)
                        if k < K:
                            stage_group(g0, n)

            # ---- finale: acc_h = acc * sqd, transpose, BN, Linear, PReLU ----
            for g0 in range(0, NT, 4):
                n = min(4, NT - g0)
                nc.vector.tensor_tensor(
                    out=acc[:, g0:g0 + n, :], in0=acc[:, g0:g0 + n, :],
                    in1=sqd[:, g0:g0 + n].to_broadcast([128, n, 128]),
                    op=mybir.AluOpType.mult)

            accT = state.tile([128, NT, 128], F32, tag="u")  # reuse u slot
            for g0 in range(0, NT, 4):
                n = min(4, NT - g0)
                psT = psum.tile([128, 512], F32, tag="ps")
                for i in range(n):
                    nc.tensor.transpose(
                        psT[:, i * 128:(i + 1) * 128],
                        acc[:, g0 + i, :], ident[:])
                nc.scalar.copy(
                    out=accT[:, g0:g0 + n, :], in_=psT[:, :n * 128]
                    .rearrange("p (t h) -> p t h", h=128))

            # BN statistics over real nodes only
            accT_f = accT[:].rearrange("p t h -> p (t h)")
            NCH = (SHARD + 511) // 512
            sums = state.tile([128, NCH, 2], F32, tag="sums")
            sqscr = tmp.tile([128, 512], F32, tag="sqscr")
            for ci in range(NCH):
                c0 = ci * 512
                cl = min(512, SHARD - c0)
                nc.scalar.activation(
                    out=sqscr[:, :cl], in_=accT_f[:, c0:c0 + cl],
                    func=mybir.ActivationFunctionType.Square,
                    accum_out=sums[:, ci, 1:2])
                nc.vector.tensor_reduce(
                    out=sums[:, ci, 0:1], in_=accT_f[:, c0:c0 + cl],
                    axis=mybir.AxisListType.X, op=mybir.AluOpType.add)
            stats = state.tile([128, 2], F32, tag="stats")
            nc.vector.tensor_reduce(
                out=stats[:, 0:1], in_=sums[:, :, 0:1],
                axis=mybir.AxisListType.XY, op=mybir.AluOpType.add)
            nc.vector.tensor_reduce(
                out=stats[:, 1:2], in_=sums[:, :, 1:2],
                axis=mybir.AxisListType.XY, op=mybir.AluOpType.add)
            nc.sync.dma_start(stats_in[:], stats[:])
            nc.gpsimd.collective_compute(
                "AllReduce", mybir.AluOpType.add,
                replica_groups=[list(range(C))],
                ins=[stats_in.opt()], outs=[stats_out.opt()],
            )
            statsg = state.tile([128, 2], F32, tag="statsg")
            nc.sync.dma_start(statsg[:], stats_out[:])
            mu = state.tile([128, 1], F32, tag="mu")
            vr = state.tile([128, 1], F32, tag="vr")
            an = state.tile([128, 1], F32, tag="an")
            bn = state.tile([128, 1], F32, tag="bn")
            nc.vector.tensor_scalar(out=mu[:], in0=statsg[:, 0:1],
                                    scalar1=1.0 / N, scalar2=None,
                                    op0=mybir.AluOpType.mult)
            nc.vector.tensor_scalar(out=vr[:], in0=statsg[:, 1:2],
                                    scalar1=1.0 / N, scalar2=None,
                                    op0=mybir.AluOpType.mult)
            mu2 = state.tile([128, 1], F32, tag="mu2")
            nc.vector.tensor_tensor(out=mu2[:], in0=mu[:], in1=mu[:],
                                    op=mybir.AluOpType.mult)
            nc.vector.tensor_tensor(out=vr[:], in0=vr[:], in1=mu2[:],
                                    op=mybir.AluOpType.subtract)
            epsv = state.tile([128, 1], F32, tag="epsv")
            nc.gpsimd.memset(epsv[:], float(EPS))
            nc.scalar.activation(out=vr[:], in_=vr[:],
                                 func=mybir.ActivationFunctionType.Sqrt,
                                 bias=epsv[:])
            nc.vector.reciprocal(vr[:], vr[:])          # rstd
            nc.vector.tensor_tensor(out=an[:], in0=vr[:], in1=bng[:],
                                    op=mybir.AluOpType.mult)
            nc.vector.tensor_tensor(out=bn[:], in0=mu[:], in1=an[:],
                                    op=mybir.AluOpType.mult)
            nc.vector.tensor_tensor(out=bn[:], in0=bnb[:], in1=bn[:],
                                    op=mybir.AluOpType.subtract)

            # normalize + Linear + PReLU + transpose back, per 512-col chunk
            y = state.tile([128, NT, OUT], F32, tag="acc")  # reuse acc slot
            for ci in range(0, NT, 4):
                n = min(4, NT - ci)
                cl = n * 128
                zc = tmp.tile([128, 512], BF16, tag="zc")
                nc.scalar.activation(
                    out=zc[:, :cl], in_=accT_f[:, ci * 128: ci * 128 + cl],
                    func=mybir.ActivationFunctionType.Identity,
                    scale=an[:], bias=bn[:])
                psy = psum.tile([128, 512], F32, tag="ps")
                nc.tensor.matmul(psy[:, :cl], lhsT=wupT[:], rhs=zc[:, :cl],
                                 start=True, stop=True)
                rl = tmp.tile([128, 512], F32, tag="rl")
                nc.scalar.activation(
                    out=rl[:, :cl], in_=psy[:, :cl],
                    func=mybir.ActivationFunctionType.Relu, bias=bup[:])
                yb = tmp.tile([128, 512], F32, tag="yb")
                nc.vector.tensor_scalar(
                    out=yb[:, :cl], in0=psy[:, :cl], scalar1=bup[:],
                    scalar2=alpha, op0=mybir.AluOpType.add,
                    op1=mybir.AluOpType.mult)
                nc.vector.tensor_scalar(
                    out=rl[:, :cl], in0=rl[:, :cl], scalar1=1.0 - alpha,
                    scalar2=None, op0=mybir.AluOpType.mult)
                nc.vector.tensor_tensor(
                    out=yb[:, :cl], in0=yb[:, :cl], in1=rl[:, :cl],
                    op=mybir.AluOpType.add)
                psT = psum.tile([128, 512], F32, tag="ps")
                for i in range(n):
                    nc.tensor.transpose(
                        psT[:, i * 128:(i + 1) * 128],
                        yb[:, i * 128: (i + 1) * 128], ident[:])
                nc.scalar.copy(
                    out=y[:, ci:ci + n, :],
                    in_=psT[:, :cl].rearrange("p (t h) -> p t h", h=128))

            # write output shard
            if FT > 0:
                nc.sync.dma_start(
                    out_t[:FT * 128, :].rearrange("(t p) h -> p t h", p=128),
                    y[:, :FT, :])
            if REM > 0:
                nc.sync.dma_start(out_t[FT * 128:, :], y[:REM, FT, :])

    nc.compile()
    return nc


# --------------------------------------------------------------------------
# Host glue
# --------------------------------------------------------------------------

def prepare_inputs(inputs, cfg, sched):
    """Build per-core in_maps from full inputs."""
    N, IN, H, OUT, C = cfg["N"], cfg["IN"], cfg["H"], cfg["OUT"], cfg["C"]
    SHARD, NT, SHARDP = sched["SHARD"], sched["NT"], sched["SHARDP"]
    x = np.asarray(inputs["x"], dtype=np.float32)
    W_enc = np.asarray(inputs["W_enc"], dtype=np.float32)
    b_enc = np.asarray(inputs["b_enc"], dtype=np.float32)
    bn_gamma = np.asarray(inputs["bn_gamma"], dtype=np.float32)
    bn_beta = np.asarray(inputs["bn_beta"], dtype=np.float32)
    W_up = np.asarray(inputs["W_up"], dtype=np.float32)
    b_up = np.asarray(inputs["b_up"], dtype=np.float32)
    dst = np.asarray(inputs["edge_index"][1], dtype=np.int64)

    deg = (np.bincount(dst, minlength=N) + 1.0).astype(np.float32)
    invdeg_full = 1.0 / deg
    isqd_full = 1.0 / np.sqrt(deg)
    sqd_full = np.sqrt(deg)

    wencT = np.ascontiguousarray(W_enc.T).astype(ml_dtypes.bfloat16)
    benc_b = np.tile(b_enc[None, :], (128, 1)).astype(np.float32)
    wupT = np.ascontiguousarray(W_up.T).astype(ml_dtypes.bfloat16)
    bup = b_up[:, None].astype(np.float32)
    bng = bn_gamma[:, None].astype(np.float32)
    bnb = bn_beta[:, None].astype(np.float32)

    in_maps = []
    for c in range(C):
        lo, hi = c * SHARD, (c + 1) * SHARD
        xp = np.zeros((SHARDP, IN), dtype=np.float32)
        xp[:SHARD] = x[lo:hi]
        xT = np.ascontiguousarray(xp.T).astype(ml_dtypes.bfloat16)

        def pad_node(v):
            p = np.ones(SHARDP, dtype=np.float32)
            p[:SHARD] = v[lo:hi]
            return np.ascontiguousarray(p.reshape(NT, 128).T)

        m = {
            "xT": xT,
            "wencT": wencT, "benc_b": benc_b, "wupT": wupT, "bup": bup,
            "bng": bng, "bnb": bnb,
            "invdeg": pad_node(invdeg_full),
            "isqd": pad_node(isqd_full),
            "sqd": pad_node(sqd_full),
            "dstl": sched["dstl"][c],
            "seed": np.zeros((1, 1), dtype=np.float32),
        }
        for qq in range(NPASS):
            arr = sched["idx_pass"][qq][c]
            if arr.shape[1] == 0:
                arr = np.zeros((128, 8), dtype=np.int16)
            m[f"idx{qq}"] = arr
        in_maps.append(m)
    return in_maps


_CACHE = {}


def run(inputs, cfg=None, n_hops=None, time_reps=0):
    """Run the model; returns (output [N, OUT] fp32, wall_exec_estimate_s)."""
    from concourse import bass2jax
    cfg = dict(CFG if cfg is None else cfg)
    C = cfg["C"]
    sched = make_schedule(np.asarray(inputs["edge_index"]), cfg["N"], C)
    gammas = np.asarray(inputs["gammas"], dtype=np.float32)
    alpha = float(np.asarray(inputs["prelu_alpha"]))
    high_pass = int(np.asarray(inputs["high_pass"]))
    nc = build(sched, cfg, gammas, alpha, high_pass, n_hops=n_hops)
    in_maps = prepare_inputs(inputs, cfg, sched)
    results = bass2jax.run_bass_via_pjrt(nc, in_maps, n_cores=C)
    out = np.concatenate([results[c]["out"] for c in range(C)], axis=0)
    return out, nc


def kernel(**inputs) -> np.ndarray:
    out, _ = run(inputs)
    return out



# revision 27
# speedup vs baseline: 4.2258x; 4.2258x over previous
"""PolyGCL GNN message-passing kernel for 8 Trainium2 NeuronCores.

Strategy (node sharding, hint-aligned):
  - Nodes are range-sharded across the 8 cores (shard = N/8 rows, padded to a
    multiple of 128). Per-node state is kept in the transformed space
    u = D^{-1/2} h, which makes the GCN edge weight separable:
        msg_u[d] = sum_{e: dst=d} u[src_e]          (pure 0/1 segment sum)
        adj_u(u) = invdeg * (msg_u + u)
    so the scatter matmul needs only one-hot selection tiles (no per-edge w).
  - Each hop: AllGather the bf16 u-shards into a full HBM table, dma_gather
    256B rows per edge (edge-major tiles), one-hot segment-sum matmuls on the
    tensor engine into PSUM per 128-dst window, then a fused combine on DVE.
  - Edges are bucketed by (dst window of 128, src quarter) on the host; the
    src quarter keeps gather indices within int16 range. Tile counts per
    bucket are maxed across cores so all 8 cores run one SPMD program.
  - BatchNorm uses local per-feature sums + a tiny AllReduce; encoder and
    output Linear layers are data-parallel over node shards.
"""

import numpy as np
import ml_dtypes

import concourse.bass as bass
import concourse.bacc as bacc
import concourse.tile as tile
from concourse import mybir
from concourse.masks import make_identity

F32 = mybir.dt.float32
BF16 = mybir.dt.bfloat16
I16 = mybir.dt.int16

CFG = dict(N=100000, E=1600000, IN=512, H=128, OUT=128, K=10, C=8,
           BN_EPS=1e-5)

CALLT = 24     # tiles (of 128 edges) per dma_gather call
SCH = 16       # tiles per one-hot S-build batch
NPASS = 4      # src quarters (int16 gather index range)
PAD_DSTL = 300.0  # out-of-window sentinel for padded edge slots


# --------------------------------------------------------------------------
# Host-side schedule
# --------------------------------------------------------------------------

def make_schedule(edge_index, N, C):
    """Bucket edges by (core, dst-window, src-quarter), pad each bucket to a
    multiple of 128 using the max count across cores (one shared SPMD
    schedule), and emit per-core gather-index / dst-slot arrays."""
    src = np.asarray(edge_index[0], dtype=np.int64)
    dst = np.asarray(edge_index[1], dtype=np.int64)
    SHARD = N // C
    NT = -(-SHARD // 128)          # node tiles == dst windows per core
    SHARDP = NT * 128
    QSP = SHARDP * C // NPASS      # quarter size in padded-global ids
    assert QSP <= 32768

    core = dst // SHARD
    local = dst - core * SHARD
    w = local // 128
    dslot = local - w * 128
    srcp = (src // SHARD) * SHARDP + (src % SHARD)   # padded-global id
    q = srcp // QSP
    idxval = srcp - q * QSP

    # counts per (core, w, q)
    key = (core * NT + w) * NPASS + q
    counts = np.bincount(key, minlength=C * NT * NPASS).reshape(C, NT, NPASS)
    T = -(-counts.max(axis=0) // 128)                 # [NT, NPASS] tiles
    nowin = T.sum(axis=1) == 0
    T[nowin, 0] = 1
    TT = int(T.sum())

    # base slot (in edges) of each (w, q) bucket, w-major q-minor
    sizes = (T * 128).reshape(-1)
    base = np.zeros(NT * NPASS, dtype=np.int64)
    base[1:] = np.cumsum(sizes)[:-1]
    base = base.reshape(NT, NPASS)

    # per-tile metadata in global (w-major, q-minor) order
    tile_w = np.repeat(np.arange(NT), T.sum(axis=1))
    tile_q = np.concatenate([np.repeat(np.arange(NPASS), T[wi]) for wi in range(NT)])
    # pass streams: position of each global tile within its pass
    pos_in_pass = np.zeros(TT, dtype=np.int64)
    for qq in range(NPASS):
        m = tile_q == qq
        pos_in_pass[m] = np.arange(m.sum())
    n_tiles_pass = [int((tile_q == qq).sum()) for qq in range(NPASS)]

    # per-core padded slot arrays
    order = np.lexsort((q, w, core))
    so_core, so_w, so_q = core[order], w[order], q[order]
    so_idx, so_dslot = idxval[order], dslot[order]
    skey = (so_core * NT + so_w) * NPASS + so_q
    # rank within each (c,w,q) group
    group_change = np.empty(len(skey), dtype=bool)
    group_change[0] = True
    group_change[1:] = skey[1:] != skey[:-1]
    gstart = np.where(group_change)[0]
    gid = np.cumsum(group_change) - 1
    rank = np.arange(len(skey)) - gstart[gid]
    slot = base[so_w, so_q] + rank

    idx_slots = np.zeros((C, TT * 128), dtype=np.int16)
    dstl_slots = np.full((C, TT * 128), PAD_DSTL, dtype=np.float32)
    idx_slots[so_core, slot] = so_idx.astype(np.int16)
    dstl_slots[so_core, slot] = so_dslot.astype(np.float32)

    # wrapped int16 index arrays per pass, per core: [128, L/16]
    idx_by_tile = idx_slots.reshape(C, TT, 128)
    idx_pass = []
    for qq in range(NPASS):
        sel = idx_by_tile[:, tile_q == qq, :].reshape(C, -1)   # [C, Lq]
        Lq = sel.shape[1]
        wrapped = sel.reshape(C, Lq // 16, 16).transpose(0, 2, 1)  # [C,16,Lq/16]
        idx_pass.append(np.tile(wrapped, (1, 8, 1)).copy())        # [C,128,..]

    dstl = np.ascontiguousarray(
        dstl_slots.reshape(C, TT, 128).transpose(0, 2, 1)
    ).astype(ml_dtypes.bfloat16)                                   # [C,128,TT]

    # window -> (first_tile, last_tile) global ids; tiles of w are contiguous
    win_first = np.zeros(NT, dtype=np.int64)
    win_last = np.zeros(NT, dtype=np.int64)
    tstart = 0
    for wi in range(NT):
        n = int(T[wi].sum())
        win_first[wi] = tstart
        win_last[wi] = tstart + n - 1
        tstart += n

    return dict(
        SHARD=SHARD, NT=NT, SHARDP=SHARDP, QSP=QSP, TT=TT,
        tile_w=tile_w, tile_q=tile_q, pos_in_pass=pos_in_pass,
        n_tiles_pass=n_tiles_pass, win_first=win_first, win_last=win_last,
        idx_pass=idx_pass, dstl=dstl,
    )


# --------------------------------------------------------------------------
# BIR builder (one SPMD program for all 8 cores)
# --------------------------------------------------------------------------

def build(sched, cfg, gammas, alpha, high_pass, n_hops=None,
          skip_collective=False, skip_gather=False, skip_matmul=False,
          skip_sbuild=False):
    N, IN, H, OUT, C = cfg["N"], cfg["IN"], cfg["H"], cfg["OUT"], cfg["C"]
    K = cfg["K"] if n_hops is None else n_hops
    EPS = cfg["BN_EPS"]
    SHARD, NT, SHARDP, QSP, TT = (sched[k] for k in
                                  ("SHARD", "NT", "SHARDP", "QSP", "TT"))
    tile_w, tile_q = sched["tile_w"], sched["tile_q"]
    pos_in_pass = sched["pos_in_pass"]
    n_tiles_pass = sched["n_tiles_pass"]
    win_first, win_last = sched["win_first"], sched["win_last"]
    KC = IN // 128
    gam = [float(gammas[i % len(gammas)]) for i in range(max((n_hops or 0) + 1, len(gammas)))]
    alpha = float(alpha)

    nc = bacc.Bacc("TRN2", target_bir_lowering=False, debug=False,
                   num_devices=C, enable_asserts=False,
                   num_swdge_queues=4, dynamic_dma_scratch_size=16384)

    # ---- I/O ----
    xT_in = nc.dram_tensor("xT", [IN, SHARDP], BF16, kind="ExternalInput")
    wencT_in = nc.dram_tensor("wencT", [IN, H], BF16, kind="ExternalInput")
    benc_in = nc.dram_tensor("benc_b", [128, H], F32, kind="ExternalInput")
    wupT_in = nc.dram_tensor("wupT", [H, OUT], BF16, kind="ExternalInput")
    bup_in = nc.dram_tensor("bup", [OUT, 1], F32, kind="ExternalInput")
    bng_in = nc.dram_tensor("bng", [H, 1], F32, kind="ExternalInput")
    bnb_in = nc.dram_tensor("bnb", [H, 1], F32, kind="ExternalInput")
    invdeg_in = nc.dram_tensor("invdeg", [128, NT], F32, kind="ExternalInput")
    isqd_in = nc.dram_tensor("isqd", [128, NT], F32, kind="ExternalInput")
    sqd_in = nc.dram_tensor("sqd", [128, NT], F32, kind="ExternalInput")
    dstl_in = nc.dram_tensor("dstl", [128, TT], BF16, kind="ExternalInput")
    idx_ins = [
        nc.dram_tensor(f"idx{qq}", [128, max(n_tiles_pass[qq] * 8, 1)], I16,
                       kind="ExternalInput")
        for qq in range(NPASS)
    ]
    seed_in = nc.dram_tensor("seed", [1, 1], F32, kind="ExternalInput")
    out_t = nc.dram_tensor("out", [SHARD, OUT], F32, kind="ExternalOutput")
    seed_out = nc.dram_tensor("seed_out", [1, 1], F32, kind="ExternalOutput")

    FT = SHARD // 128          # full node tiles
    REM = SHARD - FT * 128     # rows in the partial tile (0 => exact)

    with tile.TileContext(nc) as tc:
        with (
            tc.tile_pool(name="state", bufs=1) as state,
            tc.tile_pool(name="dram", bufs=1, space="DRAM") as dram,
            tc.tile_pool(name="mbuf", bufs=2) as mbuf,
            tc.tile_pool(name="sbld", bufs=2) as sbld,
            tc.tile_pool(name="psum", bufs=4, space="PSUM") as psum,
            tc.tile_pool(name="tmp", bufs=1) as tmp,
        ):
            # ---- persistent state ----
            u = state.tile([128, NT, H], BF16, tag="u")
            acc = state.tile([128, NT, H], F32, tag="acc")
            dstl = state.tile([128, TT], BF16, tag="dstl")
            idxs = [state.tile([128, max(n_tiles_pass[qq] * 8, 1)], I16,
                               tag=f"idx{qq}", name=f"idx{qq}_sb")
                    for qq in range(NPASS)]
            iota_bf = state.tile([128, 128], BF16, tag="iota")
            iota_f = state.tile([128, 128], F32, tag="iotaf")
            ident = state.tile([128, 128], F32, tag="ident")
            benc = state.tile([128, H], F32, tag="benc")
            wencT = state.tile([128, KC, H], BF16, tag="wencT")
            wupT = state.tile([128, OUT], BF16, tag="wupT")
            bup = state.tile([128, 1], F32, tag="bup")
            bng = state.tile([128, 1], F32, tag="bng")
            bnb = state.tile([128, 1], F32, tag="bnb")
            invdeg = state.tile([128, NT], F32, tag="invdeg")
            isqd = state.tile([128, NT], F32, tag="isqd")
            sqd = state.tile([128, NT], F32, tag="sqd")

            ag_in = dram.tile([SHARDP, H], BF16)
            ag_out = dram.tile([SHARDP * C, H], BF16)
            stats_in = dram.tile([H, 2], F32)
            stats_out = dram.tile([H, 2], F32)

            # ---- loads / constants ----
            nc.sync.dma_start(dstl[:], dstl_in[:])
            for qq in range(NPASS):
                nc.sync.dma_start(idxs[qq][:], idx_ins[qq][:])
            nc.sync.dma_start(benc[:], benc_in[:])
            nc.sync.dma_start(
                wencT[:], wencT_in[:].rearrange("(kc p) h -> p kc h", p=128))
            nc.sync.dma_start(wupT[:], wupT_in[:])
            nc.sync.dma_start(bup[:], bup_in[:])
            nc.sync.dma_start(bng[:], bng_in[:])
            nc.sync.dma_start(bnb[:], bnb_in[:])
            nc.sync.dma_start(invdeg[:], invdeg_in[:])
            nc.sync.dma_start(isqd[:], isqd_in[:])
            nc.sync.dma_start(sqd[:], sqd_in[:])
            nc.gpsimd.iota(iota_f[:], pattern=[[1, 128]], base=0,
                           channel_multiplier=0,
                           allow_small_or_imprecise_dtypes=True)
            nc.vector.tensor_copy(iota_bf[:], iota_f[:])
            make_identity(nc, ident[:])

            # seed passthrough (timing hook)
            sd = state.tile([1, 1], F32, tag="seed")
            nc.sync.dma_start(sd[:], seed_in[:])
            nc.scalar.add(sd[:], sd[:], 1.0)
            nc.sync.dma_start(seed_out[:], sd[:])

            ag_in_t = ag_in[:].rearrange("(t p) h -> p t h", p=128)

            def stage_group(g0, n):
                """u[:, g0:g0+n, :] -> ag_in (pure bf16 copy, HWDGE)."""
                nc.sync.dma_start(
                    ag_in_t[:, g0:g0 + n, :], u[:, g0:g0 + n, :])

            # ---- encoder: u0 = (x @ W_enc.T + b_enc) * isqd ----
            XCH = 1
            xT_r = xT_in[:].rearrange("(kc p) n -> p kc n", p=128)
            ps_enc = None
            for t in range(NT):
                if t % XCH == 0:
                    nx = min(XCH, NT - t) * 128
                    xc = mbuf.tile([128, KC, XCH * 128], BF16, tag="xchunk")
                    nc.sync.dma_start(
                        xc[:, :, :nx], xT_r[:, :, t * 128: t * 128 + nx])
                pt = t % 4
                if pt == 0:
                    ps_enc = psum.tile([128, 512], F32, tag="ps")
                for kc in range(KC):
                    nc.tensor.matmul(
                        ps_enc[:, pt * 128:(pt + 1) * 128],
                        lhsT=xc[:, kc, (t % XCH) * 128:(t % XCH) * 128 + 128],
                        rhs=wencT[:, kc, :],
                        start=(kc == 0), stop=(kc == KC - 1))
                if pt == 3 or t == NT - 1:
                    g0 = t - pt
                    n = pt + 1
                    psv = ps_enc[:, :n * 128].rearrange(
                        "p (t h) -> p t h", h=128)
                    t1 = tmp.tile([128, 4, 128], F32, tag="t1")
                    nc.vector.tensor_tensor(
                        out=t1[:, :n, :], in0=psv,
                        in1=benc[:].rearrange("p (o h) -> p o h", o=1)
                        .to_broadcast([128, n, 128]),
                        op=mybir.AluOpType.add)
                    nc.vector.tensor_tensor(
                        out=u[:, g0:g0 + n, :], in0=t1[:, :n, :],
                        in1=isqd[:, g0:g0 + n].to_broadcast([128, n, 128]),
                        op=mybir.AluOpType.mult)
                    nc.vector.tensor_scalar(
                        out=acc[:, g0:g0 + n, :], in0=u[:, g0:g0 + n, :],
                        scalar1=gam[0], scalar2=None,
                        op0=mybir.AluOpType.mult)
                    if K > 0:
                        stage_group(g0, n)

            # ---- hops ----
            for k in range(1, K + 1):
                if not skip_collective:
                    nc.gpsimd.collective_compute(
                        "AllGather", mybir.AluOpType.bypass,
                        replica_groups=[list(range(C))],
                        ins=[ag_in.opt()], outs=[ag_out.opt()],
                    )
                calls_emitted = [0] * NPASS
                mtiles = [None] * NPASS   # current gather output tile per pass
                msizes = [0] * NPASS
                sch_tile = None
                ps = None
                for t in range(TT):
                    wq = int(tile_q[t])
                    wi = int(tile_w[t])
                    j = int(pos_in_pass[t])
                    # gather call covering pass-tile j
                    if j // CALLT >= calls_emitted[wq]:
                        cid = calls_emitted[wq]
                        j0 = cid * CALLT
                        ntile = min(CALLT, n_tiles_pass[wq] - j0)
                        mt = mbuf.tile([128, CALLT, H], BF16, tag=f"m{wq}")
                        if not skip_gather:
                            nc.gpsimd.dma_gather(
                                out_ap=mt[:, :ntile, :],
                                in_ap=ag_out[wq * QSP:(wq + 1) * QSP, :],
                                idxs_ap=idxs[wq][:, j0 * 8: j0 * 8 + ntile * 8],
                                num_idxs=ntile * 128,
                                num_idxs_reg=ntile * 128,
                                elem_size=H,
                                single_packet=False,
                                queue_num=wq,
                            )
                        else:
                            # ablation: 1-tile gather, matmuls read slot 0
                            nc.gpsimd.dma_gather(
                                out_ap=mt[:, :1, :],
                                in_ap=ag_out[wq * QSP:(wq + 1) * QSP, :],
                                idxs_ap=idxs[wq][:, j0 * 8: j0 * 8 + 8],
                                num_idxs=128,
                                num_idxs_reg=128,
                                elem_size=H,
                                single_packet=False,
                                queue_num=wq,
                            )
                        mtiles[wq] = mt
                        msizes[wq] = ntile
                        calls_emitted[wq] += 1
                    # S chunk build
                    if t % SCH == 0:
                        nsc = min(SCH, TT - t) if not skip_sbuild else 1
                        sch_tile = sbld.tile([128, SCH, 128], BF16, tag="s")
                        nc.vector.tensor_tensor(
                            out=sch_tile[:, :nsc, :],
                            in0=dstl[:, t:t + nsc].to_broadcast([128, nsc, 128]),
                            in1=iota_bf[:].rearrange("p (o h) -> p o h", o=1)
                            .to_broadcast([128, nsc, 128]),
                            op=mybir.AluOpType.is_equal)
                    if wi % 4 == 0 and t == win_first[wi]:
                        ps = psum.tile([128, 512], F32, tag="ps")
                    sl = (wi % 4) * 128
                    s_slot = (t % SCH) if not skip_sbuild else 0
                    m_slot = (j % CALLT) if not skip_gather else 0
                    if not skip_matmul:
                        nc.tensor.matmul(
                            ps[:, sl:sl + 128],
                            lhsT=sch_tile[:, s_slot, :],
                            rhs=mtiles[wq][:, m_slot, :],
                            start=(t == win_first[wi]),
                            stop=(t == win_last[wi]))
                    elif t == win_first[wi]:
                        nc.tensor.matmul(
                            ps[:, sl:sl + 128],
                            lhsT=sch_tile[:, s_slot, :],
                            rhs=mtiles[wq][:, m_slot, :],
                            start=True, stop=True)
                    # combine at group end
                    if t == win_last[wi] and (wi % 4 == 3 or wi == NT - 1):
                        g0 = (wi // 4) * 4
                        n = wi - g0 + 1
                        psv = ps[:, :n * 128].rearrange(
                            "p (t h) -> p t h", h=128)
                        uv = u[:, g0:g0 + n, :]
                        t1 = tmp.tile([128, 4, 128], F32, tag="t1")
                        nc.vector.tensor_tensor(
                            out=t1[:, :n, :], in0=psv, in1=uv,
                            op=mybir.AluOpType.add)
                        nc.vector.tensor_tensor(
                            out=t1[:, :n, :], in0=t1[:, :n, :],
                            in1=invdeg[:, g0:g0 + n].to_broadcast(
                                [128, n, 128]),
                            op=mybir.AluOpType.mult)
                        if high_pass:
                            nc.vector.tensor_tensor(
                                out=uv, in0=uv, in1=t1[:, :n, :],
                                op=mybir.AluOpType.subtract)
                        else:
                            nc.vector.tensor_copy(uv, t1[:, :n, :])
                        t2 = tmp.tile([128, 4, 128], F32, tag="t2")
                        nc.vector.tensor_scalar(
                            out=t2[:, :n, :], in0=uv, scalar1=gam[k],
                            scalar2=None, op0=mybir.AluOpType.mult)
                        nc.vector.tensor_tensor(
                            out=acc[:, g0:g0 + n, :],
                            in0=acc[:, g0:g0 + n, :], in1=t2[:, :n, :],
                            op=mybir.AluOpType.add)
                        if k < K:
                            stage_group(g0, n)

            # ---- finale: acc_h = acc * sqd, transpose, BN, Linear, PReLU ----
            for g0 in range(0, NT, 4):
                n = min(4, NT - g0)
                nc.vector.tensor_tensor(
                    out=acc[:, g0:g0 + n, :], in0=acc[:, g0:g0 + n, :],
                    in1=sqd[:, g0:g0 + n].to_broadcast([128, n, 128]),
                    op=mybir.AluOpType.mult)

            accT = state.tile([128, NT, 128], F32, tag="u")  # reuse u slot
            for g0 in range(0, NT, 4):
                n = min(4, NT - g0)
                psT = psum.tile([128, 512], F32, tag="ps")
                for i in range(n):
                    nc.tensor.transpose(
                        psT[:, i * 128:(i + 1) * 128],
                        acc[:, g0 + i, :], ident[:])
                nc.scalar.copy(
                    out=accT[:, g0:g0 + n, :], in_=psT[:, :n * 128]
                    .rearrange("p (t h) -> p t h", h=128))

            # BN statistics over real nodes only
            accT_f = accT[:].rearrange("p t h -> p (t h)")
            NCH = (SHARD + 511) // 512
            sums = state.tile([128, NCH, 2], F32, tag="sums")
            sqscr = tmp.tile([128, 512], F32, tag="sqscr")
            for ci in range(NCH):
                c0 = ci * 512
                cl = min(512, SHARD - c0)
                nc.scalar.activation(
                    out=sqscr[:, :cl], in_=accT_f[:, c0:c0 + cl],
                    func=mybir.ActivationFunctionType.Square,
                    accum_out=sums[:, ci, 1:2])
                nc.vector.tensor_reduce(
                    out=sums[:, ci, 0:1], in_=accT_f[:, c0:c0 + cl],
                    axis=mybir.AxisListType.X, op=mybir.AluOpType.add)
            stats = state.tile([128, 2], F32, tag="stats")
            nc.vector.tensor_reduce(
                out=stats[:, 0:1], in_=sums[:, :, 0:1],
                axis=mybir.AxisListType.XY, op=mybir.AluOpType.add)
            nc.vector.tensor_reduce(
                out=stats[:, 1:2], in_=sums[:, :, 1:2],
                axis=mybir.AxisListType.XY, op=mybir.AluOpType.add)
            nc.sync.dma_start(stats_in[:], stats[:])
            nc.gpsimd.collective_compute(
                "AllReduce", mybir.AluOpType.add,
                replica_groups=[list(range(C))],
                ins=[stats_in.opt()], outs=[stats_out.opt()],
            )
            statsg = state.tile([128, 2], F32, tag="statsg")
            nc.sync.dma_start(statsg[:], stats_out[:])
            mu = state.tile([128, 1], F32, tag="mu")
            vr = state.tile([128, 1], F32, tag="vr")
            an = state.tile([128, 1], F32, tag="an")
            bn = state.tile([128, 1], F32, tag="bn")
            nc.vector.tensor_scalar(out=mu[:], in0=statsg[:, 0:1],
                                    scalar1=1.0 / N, scalar2=None,
                                    op0=mybir.AluOpType.mult)
            nc.vector.tensor_scalar(out=vr[:], in0=statsg[:, 1:2],
                                    scalar1=1.0 / N, scalar2=None,
                                    op0=mybir.AluOpType.mult)
            mu2 = state.tile([128, 1], F32, tag="mu2")
            nc.vector.tensor_tensor(out=mu2[:], in0=mu[:], in1=mu[:],
                                    op=mybir.AluOpType.mult)
            nc.vector.tensor_tensor(out=vr[:], in0=vr[:], in1=mu2[:],
                                    op=mybir.AluOpType.subtract)
            epsv = state.tile([128, 1], F32, tag="epsv")
            nc.gpsimd.memset(epsv[:], float(EPS))
            nc.scalar.activation(out=vr[:], in_=vr[:],
                                 func=mybir.ActivationFunctionType.Sqrt,
                                 bias=epsv[:])
            nc.vector.reciprocal(vr[:], vr[:])          # rstd
            nc.vector.tensor_tensor(out=an[:], in0=vr[:], in1=bng[:],
                                    op=mybir.AluOpType.mult)
            nc.vector.tensor_tensor(out=bn[:], in0=mu[:], in1=an[:],
                                    op=mybir.AluOpType.mult)
            nc.vector.tensor_tensor(out=bn[:], in0=bnb[:], in1=bn[:],
                                    op=mybir.AluOpType.subtract)

            # normalize + Linear + PReLU + transpose back, per 512-col chunk
            y = state.tile([128, NT, OUT], F32, tag="acc")  # reuse acc slot
            for ci in range(0, NT, 4):
                n = min(4, NT - ci)
                cl = n * 128
                zc = tmp.tile([128, 512], BF16, tag="zc")
                nc.scalar.activation(
                    out=zc[:, :cl], in_=accT_f[:, ci * 128: ci * 128 + cl],
                    func=mybir.ActivationFunctionType.Identity,
                    scale=an[:], bias=bn[:])
                psy = psum.tile([128, 512], F32, tag="ps")
                nc.tensor.matmul(psy[:, :cl], lhsT=wupT[:], rhs=zc[:, :cl],
                                 start=True, stop=True)
                rl = tmp.tile([128, 512], F32, tag="rl")
                nc.scalar.activation(
                    out=rl[:, :cl], in_=psy[:, :cl],
                    func=mybir.ActivationFunctionType.Relu, bias=bup[:])
                yb = tmp.tile([128, 512], F32, tag="yb")
                nc.vector.tensor_scalar(
                    out=yb[:, :cl], in0=psy[:, :cl], scalar1=bup[:],
                    scalar2=alpha, op0=mybir.AluOpType.add,
                    op1=mybir.AluOpType.mult)
                nc.vector.tensor_scalar(
                    out=rl[:, :cl], in0=rl[:, :cl], scalar1=1.0 - alpha,
                    scalar2=None, op0=mybir.AluOpType.mult)
                nc.vector.tensor_tensor(
                    out=yb[:, :cl], in0=yb[:, :cl], in1=rl[:, :cl],
                    op=mybir.AluOpType.add)
                psT = psum.tile([128, 512], F32, tag="ps")
                for i in range(n):
                    nc.tensor.transpose(
                        psT[:, i * 128:(i + 1) * 128],
                        yb[:, i * 128: (i + 1) * 128], ident[:])
                nc.scalar.copy(
                    out=y[:, ci:ci + n, :],
                    in_=psT[:, :cl].rearrange("p (t h) -> p t h", h=128))

            # write output shard
            if FT > 0:
                nc.sync.dma_start(
                    out_t[:FT * 128, :].rearrange("(t p) h -> p t h", p=128),
                    y[:, :FT, :])
            if REM > 0:
                nc.sync.dma_start(out_t[FT * 128:, :], y[:REM, FT, :])

    nc.compile()
    return nc


# --------------------------------------------------------------------------
# Host glue
# --------------------------------------------------------------------------

def prepare_inputs(inputs, cfg, sched):
    """Build per-core in_maps from full inputs."""
    N, IN, H, OUT, C = cfg["N"], cfg["IN"], cfg["H"], cfg["OUT"], cfg["C"]
    SHARD, NT, SHARDP = sched["SHARD"], sched["NT"], sched["SHARDP"]
    x = np.asarray(inputs["x"], dtype=np.float32)
    W_enc = np.asarray(inputs["W_enc"], dtype=np.float32)
    b_enc = np.asarray(inputs["b_enc"], dtype=np.float32)
    bn_gamma = np.asarray(inputs["bn_gamma"], dtype=np.float32)
    bn_beta = np.asarray(inputs["bn_beta"], dtype=np.float32)
    W_up = np.asarray(inputs["W_up"], dtype=np.float32)
    b_up = np.asarray(inputs["b_up"], dtype=np.float32)
    dst = np.asarray(inputs["edge_index"][1], dtype=np.int64)

    deg = (np.bincount(dst, minlength=N) + 1.0).astype(np.float32)
    invdeg_full = 1.0 / deg
    isqd_full = 1.0 / np.sqrt(deg)
    sqd_full = np.sqrt(deg)

    wencT = np.ascontiguousarray(W_enc.T).astype(ml_dtypes.bfloat16)
    benc_b = np.tile(b_enc[None, :], (128, 1)).astype(np.float32)
    wupT = np.ascontiguousarray(W_up.T).astype(ml_dtypes.bfloat16)
    bup = b_up[:, None].astype(np.float32)
    bng = bn_gamma[:, None].astype(np.float32)
    bnb = bn_beta[:, None].astype(np.float32)

    in_maps = []
    for c in range(C):
        lo, hi = c * SHARD, (c + 1) * SHARD
        xp = np.zeros((SHARDP, IN), dtype=np.float32)
        xp[:SHARD] = x[lo:hi]
        xT = np.ascontiguousarray(xp.T).astype(ml_dtypes.bfloat16)

        def pad_node(v):
            p = np.ones(SHARDP, dtype=np.float32)
            p[:SHARD] = v[lo:hi]
            return np.ascontiguousarray(p.reshape(NT, 128).T)

        m = {
            "xT": xT,
            "wencT": wencT, "benc_b": benc_b, "wupT": wupT, "bup": bup,
            "bng": bng, "bnb": bnb,
            "invdeg": pad_node(invdeg_full),
            "isqd": pad_node(isqd_full),
            "sqd": pad_node(sqd_full),
            "dstl": sched["dstl"][c],
            "seed": np.zeros((1, 1), dtype=np.float32),
        }
        for qq in range(NPASS):
            arr = sched["idx_pass"][qq][c]
            if arr.shape[1] == 0:
                arr = np.zeros((128, 8), dtype=np.int16)
            m[f"idx{qq}"] = arr
        in_maps.append(m)
    return in_maps


_CACHE = {}


def run(inputs, cfg=None, n_hops=None, time_reps=0):
    """Run the model; returns (output [N, OUT] fp32, wall_exec_estimate_s)."""
    from concourse import bass2jax
    cfg = dict(CFG if cfg is None else cfg)
    C = cfg["C"]
    sched = make_schedule(np.asarray(inputs["edge_index"]), cfg["N"], C)
    gammas = np.asarray(inputs["gammas"], dtype=np.float32)
    alpha = float(np.asarray(inputs["prelu_alpha"]))
    high_pass = int(np.asarray(inputs["high_pass"]))
    nc = build(sched, cfg, gammas, alpha, high_pass, n_hops=n_hops)
    in_maps = prepare_inputs(inputs, cfg, sched)
    results = bass2jax.run_bass_via_pjrt(nc, in_maps, n_cores=C)
    out = np.concatenate([results[c]["out"] for c in range(C)], axis=0)
    return out, nc


def kernel(**inputs) -> np.ndarray:
    out, _ = run(inputs)
    return out

